# revision 10
# baseline (speedup 1.0000x reference)
"""Complex-valued scaled-dot-product attention with MagMinMax normalization,
on 8 Trainium2 NeuronCores (Bass/Tile).

Math (per batch b, head h; S=1024, D=64):
  attn = (q/T) @ k^T  (complex, unconjugated)      [S, S]
  mag  = |attn|; mn/mx = min/max over key axis
  attn' = attn * (mag - mn) / ((mx - mn) * mag)
  out   = attn' @ v  (complex), returned as [2, B, H, S, D] fp32.

The normalization is scale-invariant, so the temperature divide is dropped.
Per row q:  G = (mag-mn)/((mx-mn)*mag) = a*(pst - t)  EXACTLY, where
  t   = 1/mag  (per element), pst = 1/mn = max(t), qst = 1/mx = min(t),
  a   = qst / (pst - qst).
Engine mapping per [128q x 1024k] tile (64 tiles/core):
  PE    : QK matmuls (bf16), AV matmuls (bf16)
  Act   : drain pr (PSUM->SBUF bf16), t = Sqrt(A*u), pst_i = Sqrt(A*maxu_i)
  Pool  : drain pi, output oT drain
  DVE   : custom u ~ C/mag^2 pass with MAX-accum (-> maxu -> pst),
          g1 = (t - pst) TensorScalar with MIN-accum (-> qst - pst, i.e. the
          second stat rides the affine pass for free),
          g2 = g1 * a (TensorScalar, 4x mode),
          rp|ip = ri * bcast(g2) (one paged TensorTensor, 2x mode)
  g2 = a*(t-pst) = -G; the sign is absorbed by negating v in prepack.
Stats cancellation note: qst = pst + min(t-pst) is exact in f32 because the
accumulator taps the pre-bf16-rounding datapath.

Sharding: batch dim (B=8) across the 8 cores; all heads local per core.
"""

import numpy as np

import concourse.bass as bass
import concourse.bacc as bacc
import concourse.mybir as mybir
import concourse.tile as tile
from concourse.bass_utils import run_bass_kernel_spmd

# ---------------------------------------------------------------- constants
B, H, S, D = 8, 8, 1024, 64
P = 128                 # SBUF partitions
NQT = S // P            # q tiles per head
NKB = S // P            # k blocks per head
F32 = mybir.dt.float32
BF16 = mybir.dt.bfloat16

# one-Newton reciprocal from the ~bits seed: u = z*(c0 - s*z), z = bitcast(~s)
# gives u ~ (1/A)/s with equioscillating rel err +-0.17% for c0=-8.5,
# A = 2/(18+18.0625).  sqrt(A*u) ~ 1/sqrt(s).
RECIP_C0 = -8.5
A_SCALE = 2.0 / (18.0 + 18.0625)
FLT_MAX = 3.4e38

# ------------------------------------------------------- custom DVE ops
_REGISTERED = {}


def _register_custom_ops():
    if _REGISTERED:
        return _REGISTERED
    import concourse.dve_ops as dve_ops
    from concourse.dve_spec import (
        Spec, Src0, Src1, C0, C2, Bin, AluOp, maxx, minn, lower, _has_src1,
    )
    from concourse.dve_uop import DveOpSpec

    _s = Src0 * Src0 + Src1 * Src1
    _z = Bin(AluOp.BITWISE_NOT, _s, _s)
    _y = (C0 - _s * _z) * _z

    def _mkref(np_op):
        def _ref(in0, in1, s0, s1, imm2):
            s = (in0.astype(np.float32) ** 2 + in1.astype(np.float32) ** 2
                 ).astype(np.float32)
            z = (~s.view(np.int32)).view(np.float32)
            y = ((np.float32(s0) - s * z) * z).astype(np.float32)
            acc = np_op(
                np_op.reduce(y.reshape(y.shape[0], -1), axis=-1, keepdims=True),
                np.float32(imm2))
            return y, acc
        return _ref

    specs = {
        "MAG2RECIP_MAX": Spec(body=_y, accum=maxx, accum_init=C2,
                              reference=_mkref(np.maximum)),
        "MAG2RECIP_MIN": Spec(body=_y, accum=minn, accum_init=C2,
                              reference=_mkref(np.minimum)),
    }
    for name, spec in specs.items():
        if name in dve_ops._SUB_OPCODE_FOR_NAME:
            _REGISTERED[name] = next(o for o in dve_ops.OPS if o.name == name)
            continue
        row = dve_ops._CUSTOM_DVE_ROW_BASE + len(dve_ops.OPS)
        op = dve_ops.DveOp(name, spec, False, {})
        dve_ops._SUB_OPCODE_FOR_NAME[name] = row
        for ver in ("v3", "v4"):
            uops = lower(spec, ver=ver)
            op.uops_sha[ver] = DveOpSpec(
                name=name, opcode=row, uops=uops,
                rd1_en=_has_src1(spec)).sha(ver)
        dve_ops.OPS.append(op)
        dve_ops.CUSTOM_DVE_SPECS[name] = spec
        _REGISTERED[name] = op
    return _REGISTERED


# ------------------------------------------------------------ program build
def build_nc(n_pairs=H, rep=1, finalize=True, *, dma_q="sp", ri_bufs=NQT + 4,
             g1_bufs=NQT + 2, small_bufs=4, rp_bufs=4, nact=5):
    ops = _register_custom_ops()
    op_max = ops["MAG2RECIP_MAX"]

    nc = bacc.Bacc(None, target_bir_lowering=False)
    ins = {}
    # host-prepacked, bf16:
    #   qkT[h] = [qcatT | kcatTr | kcatTi]  [P, 3S]
    #     qcatT = [qr|qi]^T, kcatTr = [kr|-ki]^T, kcatTi = [ki|kr]^T
    #   vv[h, p, j, :] = [-vcat[j*P+p] | -vcatn[j*P+p]]  [P, NKB, 2P]
    #     vcat = [vr|vi], vcatn = [-vi|vr]
    ins["qkT"] = nc.dram_tensor("qkT", (n_pairs, P, 3 * S), BF16,
                                kind="ExternalInput")
    ins["vv"] = nc.dram_tensor("vv", (n_pairs, P, NKB, 2 * P), BF16,
                               kind="ExternalInput")
    # transposed output: outt[h, 0:64, q] = out_r[h, q, :].T,
    #                    outt[h, 64:128, q] = out_i[h, q, :].T   (bf16)
    outt = nc.dram_tensor("outt", (n_pairs, P, S), BF16, kind="ExternalOutput")

    sub = mybir.AluOpType.subtract
    mult = mybir.AluOpType.mult
    amin = mybir.AluOpType.min
    sqrt_f = mybir.ActivationFunctionType.Sqrt

    with tile.TileContext(nc) as tc:
        import contextlib
        with contextlib.ExitStack() as ctx:
            pool = lambda name, bufs, **kw: ctx.enter_context(
                tc.tile_pool(name=name, bufs=bufs, **kw))
            cat_p = pool("cat", 2)              # per-pair qkT/vv bf16
            ri_p = pool("ri", ri_bufs)          # drained r|i bf16, live whole pair
            u_p = pool("u", small_bufs)
            t_p = pool("t", small_bufs)
            g1_p = pool("g1", g1_bufs)          # g1 live across the stats barrier
            g2_p = pool("g2", small_bufs)
            rp_p = pool("rp", rp_bufs or small_bufs)
            rt_p = pool("rt", 2)                # transposed rp|ip, per pair
            st_p = pool("stats", 2)
            o_p = pool("o", 2)
            psqk = pool("psqk", 3, space="PSUM")   # 3 x 2 banks
            psav = pool("psav", 1, space="PSUM")   # 1 x 2 banks

            dma_eng = {"act": nc.scalar, "sp": nc.sync, "pool": nc.gpsimd,
                       "dve": nc.vector}[dma_q]

            def body(_i=None):
                for h in range(n_pairs):
                    qkT = cat_p.tile([P, 3 * S], BF16, tag="qkT")
                    vv = cat_p.tile([P, NKB, 2 * P], BF16, tag="vv")
                    dma_eng.dma_start(out=qkT, in_=ins["qkT"][h])
                    dma_eng.dma_start(out=vv, in_=ins["vv"][h])

                    maxu = st_p.tile([P, NQT], F32, tag="maxu")
                    pstr = st_p.tile([P, NQT], F32, tag="pstr")
                    dstn = st_p.tile([P, NQT], F32, tag="dstn")
                    ri_tiles, g1_tiles, u_tiles = [], [], []

                    def stage2(j):
                        # custom u (DVE) -> pst (Act, tiny) -> sqrt t (Act)
                        # -> g1 (DVE); skewed one tile behind QK/drains so no
                        # engine blocks on another within the same tile.
                        u_t = u_p.tile([P, S], BF16, tag="u")
                        ri = ri_tiles[j]
                        nc.vector._custom_dve(
                            op_max, out=u_t, in0=ri[:, 0, :], in1=ri[:, 1, :],
                            s0=RECIP_C0, s1=0.0, imm2=-FLT_MAX,
                            accum_out=maxu[:, j:j + 1])
                        nc.scalar.activation(out=pstr[:, j:j + 1],
                                             in_=maxu[:, j:j + 1], func=sqrt_f,
                                             scale=float(A_SCALE))
                        t_t = t_p.tile([P, S], BF16, tag="t")
                        nc.scalar.activation(out=t_t, in_=u_t, func=sqrt_f,
                                             scale=float(A_SCALE))
                        g1 = g1_p.tile([P, S], BF16, tag="g1")
                        # out = t - pst_j ; accum = min(out) = qst - pst (f32)
                        nc.vector.tensor_scalar(
                            out=g1, in0=t_t, scalar1=pstr[:, j:j + 1],
                            scalar2=float(FLT_MAX), op0=sub, op1=amin,
                            accum_out=dstn[:, j:j + 1])
                        g1_tiles.append(g1)

                    # ---- QK + drains per q-tile; stage2 skewed one behind
                    for i in range(NQT):
                        qs = slice(i * P, (i + 1) * P)
                        pr = psqk.tile([P, S], F32, tag="psqk")
                        pi = psqk.tile([P, S], F32, tag="psqk")
                        for half in range(2):
                            hs = slice(half * 512, (half + 1) * 512)
                            nc.tensor.matmul(
                                pr[:, hs], qkT[:, qs],
                                qkT[:, S + half * 512:S + (half + 1) * 512],
                                start=True, stop=True)
                            nc.tensor.matmul(
                                pi[:, hs], qkT[:, qs],
                                qkT[:, 2 * S + half * 512:2 * S + (half + 1) * 512],
                                start=True, stop=True)
                        ri = ri_p.tile([P, 2, S], BF16, tag="ri")
                        nc.scalar.copy(out=ri[:, 0, :], in_=pr)
                        if i < nact:
                            nc.scalar.copy(out=ri[:, 1, :], in_=pi)
                        else:
                            nc.vector.tensor_copy(out=ri[:, 1, :], in_=pi)
                        ri_tiles.append(ri)
                        if i >= 1:
                            stage2(i - 1)
                    stage2(NQT - 1)

                    # ---- per-pair row stats -> a = qst/(pst-qst)
                    nd = st_p.tile([P, NQT], F32, tag="nd")
                    nc.vector.tensor_scalar(out=nd, in0=dstn, scalar1=-1.0,
                                            scalar2=None, op0=mult)
                    rd = st_p.tile([P, NQT], F32, tag="rd")
                    nc.vector.reciprocal(out=rd, in_=nd)
                    qst = st_p.tile([P, NQT], F32, tag="qst")
                    nc.vector.tensor_add(out=qst, in0=pstr, in1=dstn)
                    a_t = st_p.tile([P, NQT], F32, tag="a")
                    nc.vector.tensor_mul(out=a_t, in0=qst, in1=rd)

                    # ---- apply G and transpose
                    rT_all = rt_p.tile([P, 2 * NKB, S], BF16, tag="rT")
                    for i in range(NQT):
                        g2 = g2_p.tile([P, S], BF16, tag="g2")
                        nc.gpsimd.tensor_scalar(
                            out=g2, in0=g1_tiles[i], scalar1=a_t[:, i:i + 1],
                            scalar2=None, op0=mult)
                        rp = rp_p.tile([P, 2, S], BF16, tag="rp")
                        nc.vector.tensor_tensor(
                            out=rp, in0=ri_tiles[i],
                            in1=g2.unsqueeze(1).broadcast_to([P, 2, S]),
                            op=mult)
                        nc.sync.dma_start_transpose(
                            rT_all[:, :, i * P:(i + 1) * P],
                            rp.rearrange("p a b -> p (a b)"))

                    # ---- AV: outT[d2, q] += sum_j V_j^T @ A'T_j
                    oT = psav.tile([P, S], F32, tag="psav")
                    for half in range(2):
                        hs = slice(half * 512, (half + 1) * 512)
                        for j in range(NKB):
                            nc.tensor.matmul(oT[:, hs], vv[:, j, 0:P],
                                             rT_all[:, j, hs],
                                             start=(j == 0), stop=False)
                        for j in range(NKB):
                            nc.tensor.matmul(oT[:, hs], vv[:, j, P:2 * P],
                                             rT_all[:, NKB + j, hs],
                                             start=False, stop=(j == NKB - 1))
                    oT_sb = o_p.tile([P, S], BF16, tag="o")
                    nc.scalar.copy(out=oT_sb, in_=oT)
                    dma_eng.dma_start(out=outt[h], in_=oT_sb)

            if rep == 1:
                body()
            else:
                # branch-prefetch hints: the body far exceeds one IRAM block
                # per engine, so the back-edge would I$-miss (~4us/engine)
                hints = (mybir.EngineType.PE, mybir.EngineType.Activation,
                         mybir.EngineType.DVE, mybir.EngineType.Pool,
                         mybir.EngineType.SP)
                with tc.For_i(0, rep, 1, hint_engines=hints) as _i:
                    body(_i)

    if finalize:
        nc.finalize()
    else:
        nc.compile()
    return nc


# ------------------------------------------------------------- host wrapper
_NC_CACHE = {}


def _get_nc(rep=1):
    if rep not in _NC_CACHE:
        _NC_CACHE[rep] = build_nc(H, rep)
    return _NC_CACHE[rep]


def prepack(q_r, q_i, k_r, k_i, v_r, v_i):
    """Host-side layout prep: concat/transpose/tile, cast bf16."""
    import ml_dtypes
    bf16 = np.dtype(ml_dtypes.bfloat16)
    f32 = np.float32

    def catT(a, b):
        c = np.concatenate([np.asarray(a, f32), np.asarray(b, f32)],
                           axis=-1).astype(bf16)
        return np.swapaxes(c, -1, -2)

    qkT = np.ascontiguousarray(np.concatenate(
        [catT(q_r, q_i),
         catT(k_r, -np.asarray(k_i, f32)),
         catT(k_i, k_r)], axis=-1))

    # vv[..., p, j, :] = [-vcat | -vcatn] of key row j*P+p
    vcat = np.concatenate([-np.asarray(v_r, f32), -np.asarray(v_i, f32)],
                          axis=-1)
    vcatn = np.concatenate([np.asarray(v_i, f32), -np.asarray(v_r, f32)],
                           axis=-1)
    vvf = np.concatenate([vcat, vcatn], axis=-1).astype(bf16)  # [..,S,4D]
    shp = vvf.shape[:-2]
    vvf = vvf.reshape(*shp, NKB, P, 4 * D)
    vv = np.ascontiguousarray(np.moveaxis(vvf, -3, -2))        # [..,P,NKB,4D]

    return {"qkT": qkT, "vv": vv}


def kernel(q_r, q_i, k_r, k_i, v_r, v_i):
    nc = _get_nc()
    packed = prepack(q_r, q_i, k_r, k_i, v_r, v_i)
    in_maps = [{nm: np.ascontiguousarray(a[c]) for nm, a in packed.items()}
               for c in range(B)]
    res = run_bass_kernel_spmd(nc, in_maps, core_ids=list(range(B)))
    return unpack_out([res.results[c]["outt"] for c in range(B)])


def unpack_out(outts):
    out = np.empty((2, B, H, S, D), np.float32)
    for c in range(B):
        ot = np.asarray(outts[c], np.float32)       # [H, 128, S]
        out[0, c] = ot[:, 0:D, :].transpose(0, 2, 1)
        out[1, c] = ot[:, D:P, :].transpose(0, 2, 1)
    return out


if __name__ == "__main__":
    rng = np.random.default_rng(0)
    xs = {nm: rng.standard_normal((B, H, S, D), dtype=np.float32)
          for nm in ("q_r", "q_i", "k_r", "k_i", "v_r", "v_i")}
    out = kernel(**xs)
    print("kernel output", out.shape, out.dtype, float(np.abs(out).max()))


# revision 28
# speedup vs baseline: 3.9231x; 3.9231x over previous
"""Complex-valued scaled-dot-product attention with MagMinMax normalization,
on 8 Trainium2 NeuronCores (Bass/Tile).

Math (per batch b, head h; S=1024, D=64):
  attn = (q/T) @ k^T  (complex, unconjugated)      [S, S]
  mag  = |attn|; mn/mx = min/max over key axis
  attn' = attn * (mag - mn) / ((mx - mn) * mag)
  out   = attn' @ v  (complex), returned as [2, B, H, S, D] fp32.

The normalization is scale-invariant, so the temperature divide is dropped.
Per row q:  G = (mag-mn)/((mx-mn)*mag) = a*(pst - t)  EXACTLY, where
  t   = 1/mag  (per element), pst = 1/mn = max(t), qst = 1/mx = min(t),
  a   = qst / (pst - qst).
Engine mapping per [128q x 1024k] tile (64 tiles/core):
  PE    : QK matmuls (bf16), AV matmuls (bf16)
  Act   : drain pr (PSUM->SBUF bf16), t = Sqrt(A*u), pst_i = Sqrt(A*maxu_i)
  Pool  : drain pi, output oT drain
  DVE   : custom u ~ C/mag^2 pass with MAX-accum (-> maxu -> pst),
          g1 = (t - pst) TensorScalar with MIN-accum (-> qst - pst, i.e. the
          second stat rides the affine pass for free),
          g2 = g1 * a (TensorScalar, 4x mode),
          rp|ip = ri * bcast(g2) (one paged TensorTensor, 2x mode)
  g2 = a*(t-pst) = -G; the sign is absorbed by negating v in prepack.
Stats cancellation note: qst = pst + min(t-pst) is exact in f32 because the
accumulator taps the pre-bf16-rounding datapath.

Sharding: batch dim (B=8) across the 8 cores; all heads local per core.
"""

import numpy as np

import concourse.bass as bass
import concourse.bacc as bacc
import concourse.mybir as mybir
import concourse.tile as tile
from concourse.bass_utils import run_bass_kernel_spmd

# ---------------------------------------------------------------- constants
B, H, S, D = 8, 8, 1024, 64
P = 128                 # SBUF partitions
NQT = S // P            # q tiles per head
NKB = S // P            # k blocks per head
F32 = mybir.dt.float32
BF16 = mybir.dt.bfloat16

# one-Newton reciprocal from the ~bits seed: u = z*(c0 - s*z), z = bitcast(~s)
# gives u ~ (1/A)/s with equioscillating rel err +-0.17% for c0=-8.5,
# A = 2/(18+18.0625).  sqrt(A*u) ~ 1/sqrt(s).
RECIP_C0 = -8.5
A_SCALE = 2.0 / (18.0 + 18.0625)
FLT_MAX = 3.4e38

# ------------------------------------------------------- custom DVE ops
_REGISTERED = {}


def _register_custom_ops():
    if _REGISTERED:
        return _REGISTERED
    import concourse.dve_ops as dve_ops
    from concourse.dve_spec import (
        Spec, Src0, Src1, C0, C2, Bin, AluOp, maxx, minn, lower, _has_src1,
    )
    from concourse.dve_uop import DveOpSpec

    _s = Src0 * Src0 + Src1 * Src1
    _z = Bin(AluOp.BITWISE_NOT, _s, _s)
    _y = (C0 - _s * _z) * _z

    def _mkref(np_op):
        def _ref(in0, in1, s0, s1, imm2):
            s = (in0.astype(np.float32) ** 2 + in1.astype(np.float32) ** 2
                 ).astype(np.float32)
            z = (~s.view(np.int32)).view(np.float32)
            y = ((np.float32(s0) - s * z) * z).astype(np.float32)
            acc = np_op(
                np_op.reduce(y.reshape(y.shape[0], -1), axis=-1, keepdims=True),
                np.float32(imm2))
            return y, acc
        return _ref

    specs = {
        "MAG2RECIP_MAX": Spec(body=_y, accum=maxx, accum_init=C2,
                              reference=_mkref(np.maximum)),
        "MAG2RECIP_MIN": Spec(body=_y, accum=minn, accum_init=C2,
                              reference=_mkref(np.minimum)),
    }
    for name, spec in specs.items():
        if name in dve_ops._SUB_OPCODE_FOR_NAME:
            _REGISTERED[name] = next(o for o in dve_ops.OPS if o.name == name)
            continue
        row = dve_ops._CUSTOM_DVE_ROW_BASE + len(dve_ops.OPS)
        op = dve_ops.DveOp(name, spec, False, {})
        dve_ops._SUB_OPCODE_FOR_NAME[name] = row
        for ver in ("v3", "v4"):
            uops = lower(spec, ver=ver)
            op.uops_sha[ver] = DveOpSpec(
                name=name, opcode=row, uops=uops,
                rd1_en=_has_src1(spec)).sha(ver)
        dve_ops.OPS.append(op)
        dve_ops.CUSTOM_DVE_SPECS[name] = spec
        _REGISTERED[name] = op
    return _REGISTERED


# ------------------------------------------------------------ program build
def build_nc(n_pairs=H, rep=1, finalize=True, *, dma_q="sp", ri_bufs=7,
             g1_bufs=3, small_bufs=3, rp_bufs=2, nact=5,
             no_accum=False, split_mul=False, batch_pst=False, split_tr=False,
             g2_dve=True, ablate=(), statv="accum"):
    ops = _register_custom_ops()
    op_max = ops["MAG2RECIP_MAX"]
    op_min = ops["MAG2RECIP_MIN"]

    nc = bacc.Bacc(None, target_bir_lowering=False)
    ins = {}
    # host-prepacked, bf16:
    #   qkT[h] = [qcatT | kcatTr | kcatTi]  [P, 3S]
    #     qcatT = [qr|qi]^T, kcatTr = [kr|-ki]^T, kcatTi = [ki|kr]^T
    #   vv[h, p, j, :] = [-vcat[j*P+p] | -vcatn[j*P+p]]  [P, NKB, 2P]
    #     vcat = [vr|vi], vcatn = [-vi|vr]
    ins["qkT"] = nc.dram_tensor("qkT", (n_pairs, P, 3 * S), BF16,
                                kind="ExternalInput")
    ins["vv"] = nc.dram_tensor("vv", (n_pairs, P, NKB, 2 * P), BF16,
                               kind="ExternalInput")
    # transposed output: outt[h, 0:64, q] = out_r[h, q, :].T,
    #                    outt[h, 64:128, q] = out_i[h, q, :].T   (bf16)
    outt = nc.dram_tensor("outt", (n_pairs, P, S), BF16, kind="ExternalOutput")

    sub = mybir.AluOpType.subtract
    mult = mybir.AluOpType.mult
    amin = mybir.AluOpType.min
    sqrt_f = mybir.ActivationFunctionType.Sqrt

    with tile.TileContext(nc) as tc:
        import contextlib
        with contextlib.ExitStack() as ctx:
            pool = lambda name, bufs, **kw: ctx.enter_context(
                tc.tile_pool(name=name, bufs=bufs, **kw))
            cat_p = pool("cat", 2)              # per-pair qkT/vv bf16
            ri_p = pool("ri", ri_bufs)          # drained r|i bf16, live whole pair
            u_p = pool("u", small_bufs)
            t_p = pool("t", small_bufs)
            g1_p = pool("g1", g1_bufs)          # g1 live across the stats barrier
            g2_p = pool("g2", small_bufs)
            rp_p = pool("rp", rp_bufs or small_bufs)
            rt_p = pool("rt", 3)                # transposed rp|ip, per pair
            st_p = pool("stats", 2)
            o_p = pool("o", 2)
            psqk = pool("psqk", 3, space="PSUM")   # 3 x 2 banks
            psav = pool("psav", 1, space="PSUM")   # 1 x 2 banks

            dma_eng = {"act": nc.scalar, "sp": nc.sync, "pool": nc.gpsimd,
                       "dve": nc.vector}[dma_q]

            def body(_i=None):
                for h in range(n_pairs):
                    qkT = cat_p.tile([P, 3 * S], BF16, tag="qkT")
                    vv = cat_p.tile([P, NKB, 2 * P], BF16, tag="vv")
                    dma_eng.dma_start(out=qkT, in_=ins["qkT"][h])
                    dma_eng.dma_start(out=vv, in_=ins["vv"][h])

                    maxu = st_p.tile([P, NQT], F32, tag="maxu")
                    pstr = st_p.tile([P, NQT], F32, tag="pstr")
                    qstr = st_p.tile([P, NQT], F32, tag="qstr")
                    dstn = st_p.tile([P, NQT], F32, tag="dstn")
                    rT_grp = [None, None]
                    rp4 = [None]
                    ri_tiles, g1_tiles, u_tiles, t_tiles_abl = [], [], [], []

                    def stage2(j):
                        # custom u (DVE) -> pst (Act, tiny) -> sqrt t (Act)
                        # -> g1 (DVE); skewed one tile behind QK/drains so no
                        # engine blocks on another within the same tile.
                        u_t = u_p.tile([P, S], BF16, tag="u")
                        ri = ri_tiles[j]
                        nc.vector._custom_dve(
                            op_min, out=u_t, in0=ri[:, 0, :], in1=ri[:, 1, :],
                            s0=RECIP_C0, s1=0.0, imm2=FLT_MAX,
                            accum_out=maxu[:, j:j + 1])
                        # qst = sqrt(A*minu) = 1/mx
                        nc.scalar.activation(out=qstr[:, j:j + 1],
                                             in_=maxu[:, j:j + 1],
                                             func=sqrt_f,
                                             scale=float(A_SCALE))
                        t_t = t_p.tile([P, S], BF16, tag="t")
                        if "sqrt" in ablate:
                            if j == 0:
                                nc.scalar.activation(out=t_t, in_=u_t,
                                                     func=sqrt_f,
                                                     scale=float(A_SCALE))
                                t_tiles_abl.append(t_t)
                            else:
                                t_t = t_tiles_abl[-1]
                        else:
                            nc.scalar.activation(out=t_t, in_=u_t, func=sqrt_f,
                                                 scale=float(A_SCALE))
                        # second stat: max(u) over the row
                        # (tensor_tensor_reduce crashes at runtime on this HW)
                        if statv == "accum":
                            # g1 = (t - qst); accum MAX -> pst - qst
                            g1a = g1_p.tile([P, S], BF16, tag="g1a")
                            nc.vector.tensor_scalar(
                                out=g1a, in0=t_t, scalar1=qstr[:, j:j + 1],
                                scalar2=-float(FLT_MAX), op0=sub,
                                op1=mybir.AluOpType.max,
                                accum_out=dstn[:, j:j + 1])
                        elif statv == "tr":
                            nc.vector.tensor_reduce(
                                out=dstn[:, j:j + 1], in_=u_t,
                                axis=mybir.AxisListType.X,
                                op=mybir.AluOpType.max)
                        elif statv == "pool":
                            nc.gpsimd.tensor_reduce(
                                out=dstn[:, j:j + 1], in_=u_t,
                                axis=mybir.AxisListType.X,
                                op=mybir.AluOpType.max)
                        elif statv == "accum":
                            pass  # handled below via g1-accum
                        elif statv == "tree":
                            m1 = u_p.tile([P, S // 2], BF16, tag="m1")
                            nc.vector.tensor_max(out=m1, in0=u_t[:, 0:S // 2],
                                                 in1=u_t[:, S // 2:S])
                            nc.vector.tensor_reduce(
                                out=dstn[:, j:j + 1], in_=m1,
                                axis=mybir.AxisListType.X,
                                op=mybir.AluOpType.max)
                        # pst = sqrt(A*maxu) = 1/mn (Act small, pipelined free)
                        if statv == "accum":
                            nc.vector.tensor_add(out=pstr[:, j:j + 1],
                                                 in0=qstr[:, j:j + 1],
                                                 in1=dstn[:, j:j + 1])
                        else:
                            nc.scalar.activation(out=pstr[:, j:j + 1],
                                                 in_=dstn[:, j:j + 1],
                                                 func=sqrt_f,
                                                 scale=float(A_SCALE))
                        # a' = qst * recip(qst - pst)  (= -a; sign folds into
                        # the un-negated v prepack)
                        dsn = st_p.tile([P, NQT], F32, tag="dsn")
                        rdn = st_p.tile([P, NQT], F32, tag="rdn")
                        a_t = st_p.tile([P, NQT], F32, tag="a")
                        nc.vector.tensor_sub(out=dsn[:, j:j + 1],
                                             in0=qstr[:, j:j + 1],
                                             in1=pstr[:, j:j + 1])
                        nc.vector.reciprocal(out=rdn[:, j:j + 1],
                                             in_=dsn[:, j:j + 1])
                        nc.vector.tensor_mul(out=a_t[:, j:j + 1],
                                             in0=qstr[:, j:j + 1],
                                             in1=rdn[:, j:j + 1])
                        g2 = g2_p.tile([P, S], BF16, tag="g2")
                        if statv == "accum":
                            # g1 = t - qst with MIN?? no: g1 = t - qst ; accum
                            # max(t - qst) = pst - qst ; then g2 = g1 * a''
                            nc.vector.tensor_scalar(
                                out=g2, in0=t_t, scalar1=pstr[:, j:j + 1],
                                scalar2=a_t[:, j:j + 1], op0=sub, op1=mult)
                        else:
                            # g = (t-pst) * a' (single 2-scalar TSP, no accum)
                            nc.vector.tensor_scalar(
                                out=g2, in0=t_t, scalar1=pstr[:, j:j + 1],
                                scalar2=a_t[:, j:j + 1], op0=sub, op1=mult)
                        if j % 4 == 0:
                            rp4[0] = rp_p.tile([P, 4, 2, S], BF16, tag="rp", name="rp4")
                        rp = rp4[0]
                        nc.vector.tensor_tensor(
                            out=rp[:, j % 4], in0=ri_tiles[j],
                            in1=g2.unsqueeze(1).broadcast_to([P, 2, S]),
                            op=mult)
                        if j % 4 == 3:
                            g = j // 4
                            rT_grp[g] = rt_p.tile([P, 4, 2, NKB, P], BF16,
                                                  tag="rT", name="rTg")
                            nc.sync.dma_start_transpose(
                                rT_grp[g].rearrange("p t c k q -> p (t c k) q"),
                                rp.rearrange("p t c b -> p (t c b)"))

                    # ---- QK + drains per q-tile; stage2 skewed one behind
                    pr = pi = None
                    for i in range(NQT):
                        qs = slice(i * P, (i + 1) * P)
                        if "qk" in ablate and i > 0:
                            pass
                        else:
                            pr = psqk.tile([P, S], F32, tag="psqk")
                            pi = psqk.tile([P, S], F32, tag="psqk")
                        for half in (() if ("qk" in ablate and i > 0) else range(2)):
                            hs = slice(half * 512, (half + 1) * 512)
                            nc.tensor.matmul(
                                pr[:, hs], qkT[:, qs],
                                qkT[:, S + half * 512:S + (half + 1) * 512],
                                start=True, stop=True)
                            nc.tensor.matmul(
                                pi[:, hs], qkT[:, qs],
                                qkT[:, 2 * S + half * 512:2 * S + (half + 1) * 512],
                                start=True, stop=True)
                        ri = ri_p.tile([P, 2, S], BF16, tag="ri")
                        if "drain" in ablate:
                            if i == 0:
                                nc.scalar.copy(out=ri[:, 0, :], in_=pr)
                                nc.scalar.copy(out=ri[:, 1, :], in_=pi)
                            else:
                                ri = ri_tiles[0]
                        else:
                            nc.scalar.copy(out=ri[:, 0, :], in_=pr)
                            if i < nact:
                                nc.scalar.copy(out=ri[:, 1, :], in_=pi)
                            else:
                                nc.vector.tensor_copy(out=ri[:, 1, :], in_=pi)
                        ri_tiles.append(ri)
                        if i >= 1:
                            stage2(i - 1)
                    stage2(NQT - 1)

                    # ---- AV: outT[d2, q] += sum_j V_j^T @ A'T_j
                    oT = psav.tile([P, S], F32, tag="psav")
                    for half in range(2):
                        hs = slice(half * 512, (half + 1) * 512)
                        jset = range(NKB) if "av" not in ablate else [0]
                        for j in jset:
                            nc.tensor.matmul(oT[:, hs], vv[:, j, 0:P],
                                             rT_grp[half][:, :, 0, j, :],
                                             start=(j == 0), stop=False)
                        jset2 = range(NKB) if "av" not in ablate else [NKB - 1]
                        for j in jset2:
                            nc.tensor.matmul(oT[:, hs], vv[:, j, P:2 * P],
                                             rT_grp[half][:, :, 1, j, :],
                                             start=False, stop=(j == NKB - 1))
                    oT_sb = o_p.tile([P, S], BF16, tag="o")
                    nc.scalar.copy(out=oT_sb, in_=oT)
                    dma_eng.dma_start(out=outt[h], in_=oT_sb)

            if rep == 1:
                body()
            else:
                # branch-prefetch hints: the body far exceeds one IRAM block
                # per engine, so the back-edge would I$-miss (~4us/engine)
                hints = (mybir.EngineType.PE, mybir.EngineType.Activation,
                         mybir.EngineType.DVE, mybir.EngineType.Pool,
                         mybir.EngineType.SP)
                with tc.For_i(0, rep, 1, hint_engines=hints) as _i:
                    body(_i)

    if finalize:
        nc.finalize()
    else:
        nc.compile()
    return nc


# ------------------------------------------------------------- host wrapper
_NC_CACHE = {}


def _get_nc(rep=1):
    if rep not in _NC_CACHE:
        _NC_CACHE[rep] = build_nc(H, rep)
    return _NC_CACHE[rep]


def prepack(q_r, q_i, k_r, k_i, v_r, v_i):
    """Host-side layout prep: concat/transpose/tile, cast bf16."""
    import ml_dtypes
    bf16 = np.dtype(ml_dtypes.bfloat16)
    f32 = np.float32

    def catT(a, b):
        c = np.concatenate([np.asarray(a, f32), np.asarray(b, f32)],
                           axis=-1).astype(bf16)
        return np.swapaxes(c, -1, -2)

    qkT = np.ascontiguousarray(np.concatenate(
        [catT(q_r, q_i),
         catT(k_r, -np.asarray(k_i, f32)),
         catT(k_i, k_r)], axis=-1))

    # vv[..., p, j, :] = [-vcat | -vcatn] of key row j*P+p
    vcat = np.concatenate([np.asarray(v_r, f32), np.asarray(v_i, f32)],
                          axis=-1)
    vcatn = np.concatenate([-np.asarray(v_i, f32), np.asarray(v_r, f32)],
                           axis=-1)
    vvf = np.concatenate([vcat, vcatn], axis=-1).astype(bf16)  # [..,S,4D]
    shp = vvf.shape[:-2]
    vvf = vvf.reshape(*shp, NKB, P, 4 * D)
    vv = np.ascontiguousarray(np.moveaxis(vvf, -3, -2))        # [..,P,NKB,4D]

    return {"qkT": qkT, "vv": vv}


def kernel(q_r, q_i, k_r, k_i, v_r, v_i):
    nc = _get_nc()
    packed = prepack(q_r, q_i, k_r, k_i, v_r, v_i)
    in_maps = [{nm: np.ascontiguousarray(a[c]) for nm, a in packed.items()}
               for c in range(B)]
    res = run_bass_kernel_spmd(nc, in_maps, core_ids=list(range(B)))
    return unpack_out([res.results[c]["outt"] for c in range(B)])


def unpack_out(outts):
    out = np.empty((2, B, H, S, D), np.float32)
    for c in range(B):
        ot = np.asarray(outts[c], np.float32)       # [H, 128, S]
        out[0, c] = ot[:, 0:D, :].transpose(0, 2, 1)
        out[1, c] = ot[:, D:P, :].transpose(0, 2, 1)
    return out


if __name__ == "__main__":
    rng = np.random.default_rng(0)
    xs = {nm: rng.standard_normal((B, H, S, D), dtype=np.float32)
          for nm in ("q_r", "q_i", "k_r", "k_i", "v_r", "v_i")}
    out = kernel(**xs)
    print("kernel output", out.shape, out.dtype, float(np.abs(out).max()))
